# revision 9
# baseline (speedup 1.0000x reference)
"""Distributed Trainium2 kernel for the 3-branch masked attention problem.

Sharding: 8 cores; core c handles batch b = c//2 and heads h0 = 4*(c%2) .. +4
(data + head parallel).  Each core computes QKV for its heads, the three
branch softmaxes and AV locally; per-I-block (512-token) AllGathers of the
attention output let both cores of a batch apply the output projection with
the collectives overlapped under the attention tail.

Schedule (v2): the scalar engine's exp stream is the critical resource
(50.3M exps/core at 1 elem/cycle/lane = ~440us).  Everything else is
arranged to hide under it: QKV is split so attention starts as soon as
branch-a q/k and the combined V are ready (~65us), the p/k q/k projections
interleave into branch-a attention's PE slack, a third of the mask
multiplies go to the otherwise-idle gpsimd engine, and the output
collective+projection is chunked per I-block so only the last quarter
trails the final exp.
"""

import numpy as np
import ml_dtypes

BF16 = ml_dtypes.bfloat16

H = 8
DA, DP, DK = 2048, 1024, 1024
B, N = 4, 2048
DOUT = 512
H_LOC = 4           # heads per core
DA_H, DP_H, DK_H = DA // H, DP // H, DK // H      # 256, 128, 128
da, dp, dk = DA_H // H, DP_H // H, DK_H // H      # 32, 16, 16
DV = da + dp + dk                                 # 64
NCORES = 8

IB = 512            # query block (moving dim of dots / AV)
JB = 128            # key chunk (contract chunk of AV, M of dots)
NI = N // IB        # 4
NJ = N // JB        # 16

_CACHE = {}


def _build():
    import concourse.bass as bass
    import concourse.mybir as mybir
    import concourse.tile as tile
    from concourse import bacc
    from concourse.masks import make_identity
    from concourse.tile import add_dep_helper

    f32 = mybir.dt.float32
    bf16 = mybir.dt.bfloat16
    Exp = mybir.ActivationFunctionType.Exp
    mult = mybir.AluOpType.mult
    add = mybir.AluOpType.add

    nc = bacc.Bacc("TRN2", target_bir_lowering=False, debug=False,
                   enable_asserts=False, num_devices=NCORES)

    xT = nc.dram_tensor("xT", [DA + DP + DK, N], bf16, kind="ExternalInput")
    maskT = nc.dram_tensor("maskT", [N, N], bf16, kind="ExternalInput")
    waT = nc.dram_tensor("waT", [DA, 384], bf16, kind="ExternalInput")
    wpT = nc.dram_tensor("wpT", [DP, 384], bf16, kind="ExternalInput")
    wkT = nc.dram_tensor("wkT", [DK, 384], bf16, kind="ExternalInput")
    woutT = nc.dram_tensor("woutT", [DOUT, DOUT], bf16, kind="ExternalInput")
    bout = nc.dram_tensor("bout", [DOUT, 1], f32, kind="ExternalInput")
    out = nc.dram_tensor("out", [DOUT, N], bf16, kind="ExternalOutput")

    with tile.TileContext(nc) as tc:
        with (
            tc.tile_pool(name="const", bufs=1) as cpool,
            tc.tile_pool(name="dram", bufs=1, space="DRAM") as dpool,
        ):
            # ---- constants ----
            ident_bf = cpool.tile([128, 128], bf16)
            make_identity(nc, ident_bf)
            ident_f32 = cpool.tile([128, 128], f32)
            make_identity(nc, ident_f32)

            bias_sb = cpool.tile([128, 4], f32)
            for t in range(4):
                nc.sync.dma_start(bias_sb[:, t:t + 1], bout[128 * t:128 * (t + 1), :])

            wa_sb = [cpool.tile([128, 384], bf16, name=f"wa{f}") for f in range(16)]
            for f in range(16):
                nc.sync.dma_start(wa_sb[f][:], waT[128 * f:128 * (f + 1), :])
            wp_sb = [cpool.tile([128, 384], bf16, name=f"wp{f}") for f in range(8)]
            wk_sb = [cpool.tile([128, 384], bf16, name=f"wk{f}") for f in range(8)]
            for f in range(8):
                nc.sync.dma_start(wp_sb[f][:], wpT[128 * f:128 * (f + 1), :])
                nc.sync.dma_start(wk_sb[f][:], wkT[128 * f:128 * (f + 1), :])
            wo_sb = [cpool.tile([128, DOUT], bf16, name=f"wo{f}") for f in range(4)]
            for f in range(4):
                nc.sync.dma_start(wo_sb[f][:], woutT[128 * f:128 * (f + 1), :])

            # warm the exp table set off the critical path
            warm_sb = cpool.tile([128, 16], bf16)
            nc.scalar.activation(warm_sb[:], ident_f32[:, 0:16], Exp)

            # ---- persistent activations ----
            # qT/kT per branch: [128, N]; heads live at 32-aligned partition bases
            qTa = cpool.tile([128, N], bf16)
            kTa = cpool.tile([128, N], bf16)
            qTp = cpool.tile([128, N], bf16)
            kTp = cpool.tile([128, N], bf16)
            qTk = cpool.tile([128, N], bf16)
            kTk = cpool.tile([128, N], bf16)
            # V^T split by branch group so branch-a V lands before p/k V:
            # comb_va rows 32h+[0:32] = va of head h
            # comb_vpk rows 32h+[vp(16)|vk(16)]
            comb_va = cpool.tile([128, N], bf16, name="cva")
            comb_vpk = cpool.tile([128, N], bf16, name="cvpk")
            # V_aug per head: 16 chunks of [128, 128] side by side: cols
            # 0:64 = v, col 64 = ones, 65:128 = zeros
            vaug = [cpool.tile([128, 128 * NJ], bf16, name=f"vaug{h}") for h in range(H_LOC)]
            # normalized attention output accumulator, [token, dv] layout
            oacc = [[cpool.tile([128, DV], f32, name=f"oacc{h}_{s}") for s in range(N // 128)]
                    for h in range(H_LOC)]
            # final transposed attention output (this core's heads)
            otc = [cpool.tile([128, N], bf16, name=f"otc{i}") for i in range(2)]

            # per-I-block collective bounce buffers (quarter gathers)
            cc_in_q = [dpool.tile([2 * 128, IB], bf16, name=f"ccin{I}")
                       for I in range(NI)]
            cc_out_q = [dpool.tile([4 * 128, IB], bf16, name=f"ccout{I}")
                        for I in range(NI)]

            for h in range(H_LOC):
                nc.gpsimd.memset(vaug[h][:], 0.0)
                for j in range(NJ):
                    nc.gpsimd.memset(vaug[h][:, 128 * j + 64:128 * j + 65], 1.0)

            _mctx = tc.tile_pool(name="mask", bufs=1)
            mpool = _mctx.__enter__()
            m_sb = [mpool.tile([128, N], bf16, name=f"m{j}") for j in range(NJ)]
            # first mask tiles before the x stream so the attention head
            # start never races the mask DMA
            for j in range(2):
                nc.sync.dma_start(m_sb[j][:], maskT[128 * j:128 * (j + 1), :])

            # =================== QKV projection ===================
            # pass1: branch a complete (q, k, v in one x stream)
            # pass2v: p/k v projections (shared accumulator, one x stream)
            # pass3 (emitted interleaved into branch-a attention): p/k q,k
            with (
                tc.tile_pool(name="xs", bufs=8) as xpool,
                tc.tile_pool(name="qkv_ps", bufs=6, space="PSUM") as qkv_ps,
            ):
                for tp2 in range(2):
                    t0 = 2 * IB * tp2
                    ps_q = [qkv_ps.tile([128, IB], f32, tag="qkv", name=f"psq{u}")
                            for u in range(2)]
                    ps_k = [qkv_ps.tile([128, IB], f32, tag="qkv", name=f"psk{u}")
                            for u in range(2)]
                    ps_va = [qkv_ps.tile([128, IB], f32, tag="qkv", name=f"psva{u}")
                             for u in range(2)]
                    for f in range(16):
                        xt = xpool.tile([128, 2 * IB], bf16, tag="x")
                        nc.sync.dma_start(
                            xt[:], xT[128 * f:128 * (f + 1), t0:t0 + 2 * IB])
                        st, sp = (f == 0), (f == 15)
                        w = wa_sb[f]
                        for u in range(2):
                            xu = xt[:, IB * u:IB * (u + 1)]
                            nc.tensor.matmul(ps_q[u][:], w[:, 0:128], xu, start=st, stop=sp)
                            nc.tensor.matmul(ps_k[u][:], w[:, 128:256], xu, start=st, stop=sp)
                            nc.tensor.matmul(ps_va[u][:], w[:, 256:384], xu, start=st, stop=sp)
                    for u in range(2):
                        tsl = slice(t0 + IB * u, t0 + IB * (u + 1))
                        nc.vector.tensor_copy(qTa[:, tsl], ps_q[u][:])
                        nc.vector.tensor_copy(kTa[:, tsl], ps_k[u][:])
                        nc.vector.tensor_copy(comb_va[:, tsl], ps_va[u][:])

                # pass2v: p+k v projections (shared accumulator)
                for tp2 in range(2):
                    t0 = 2 * IB * tp2
                    ps_vpk = [qkv_ps.tile([128, IB], f32, tag="qkv", name=f"psvpk{u}")
                              for u in range(2)]
                    for bi, wsb in ((1, wp_sb), (2, wk_sb)):
                        fofs = DA if bi == 1 else DA + DP
                        for f in range(8):
                            xt = xpool.tile([128, 2 * IB], bf16, tag="x")
                            nc.sync.dma_start(
                                xt[:], xT[fofs + 128 * f:fofs + 128 * (f + 1),
                                          t0:t0 + 2 * IB])
                            vst = (f == 0) and bi == 1
                            vsp = (f == 7) and bi == 2
                            for u in range(2):
                                nc.tensor.matmul(ps_vpk[u][:], wsb[f][:, 256:384],
                                                 xt[:, IB * u:IB * (u + 1)],
                                                 start=vst, stop=vsp)
                    for u in range(2):
                        tsl = slice(t0 + IB * u, t0 + IB * (u + 1))
                        nc.vector.tensor_copy(comb_vpk[:, tsl], ps_vpk[u][:])

                # remaining mask tiles (behind both x streams on the queue)
                for j in range(2, NJ):
                    nc.sync.dma_start(m_sb[j][:], maskT[128 * j:128 * (j + 1), :])

            # =================== attention ===================
            with (
                tc.tile_pool(name="otf", bufs=1) as otfpool,
                tc.tile_pool(name="s_ps", bufs=2, space="PSUM") as s_ps_pool,
                tc.tile_pool(name="o_ps", bufs=2, space="PSUM") as o_ps_pool,
                tc.tile_pool(name="aux_ps", bufs=2, space="PSUM") as aux_ps_pool,
                tc.tile_pool(name="ep", bufs=4) as epool,
                tc.tile_pool(name="pp", bufs=5) as ppool,
                tc.tile_pool(name="ob", bufs=3) as opool,
                tc.tile_pool(name="rr", bufs=4) as rpool,
                tc.tile_pool(name="x3", bufs=2) as x3pool,
            ):
                # V_aug: transpose comb chunks (aux psum); transposed cols
                # 32h:32h+32 belong to head h
                for j in range(NJ):
                    jsl = slice(128 * j, 128 * (j + 1))
                    tpv = aux_ps_pool.tile([128, 128], bf16, tag="aux", name="tpv")
                    nc.tensor.transpose(tpv[:], comb_va[:, jsl], ident_bf[:])
                    for h in range(H_LOC):
                        nc.vector.tensor_copy(vaug[h][:, 128 * j:128 * j + 32],
                                              tpv[:, 32 * h:32 * h + 32])
                    tpw = aux_ps_pool.tile([128, 128], bf16, tag="aux", name="tpw")
                    nc.tensor.transpose(tpw[:], comb_vpk[:, jsl], ident_bf[:])
                    for h in range(H_LOC):
                        nc.vector.tensor_copy(vaug[h][:, 128 * j + 32:128 * j + 64],
                                              tpw[:, 32 * h:32 * h + 32])

                # pass3 work queue: p/k q,k projections, consumed in chunks
                # between branch-a attention iterations (PE slack under the
                # ACT-bound exp stream)
                def pass3_segment(bi3, tp2, u):
                    wsb = wp_sb if bi3 == 1 else wk_sb
                    fofs = DA if bi3 == 1 else DA + DP
                    t0 = 2 * IB * tp2
                    usl = slice(t0 + IB * u, t0 + IB * (u + 1))
                    ps_q3 = aux_ps_pool.tile([128, IB], f32, tag="aux", name="psq3")
                    ps_k3 = aux_ps_pool.tile([128, IB], f32, tag="aux", name="psk3")
                    for f in range(8):
                        xt = x3pool.tile([128, IB], bf16, tag="x3")
                        nc.sync.dma_start(
                            xt[:], xT[fofs + 128 * f:fofs + 128 * (f + 1), usl])
                        st, sp = (f == 0), (f == 7)
                        nc.tensor.matmul(ps_q3[:], wsb[f][:, 0:128], xt[:],
                                         start=st, stop=sp)
                        nc.tensor.matmul(ps_k3[:], wsb[f][:, 128:256], xt[:],
                                         start=st, stop=sp)
                    qT3 = qTp if bi3 == 1 else qTk
                    kT3 = kTp if bi3 == 1 else kTk
                    nc.vector.tensor_copy(qT3[:, usl], ps_q3[:])
                    nc.vector.tensor_copy(kT3[:, usl], ps_k3[:])

                pass3_work = [(bi3, tp2, u) for bi3 in (1, 2)
                              for tp2 in range(2) for u in range(2)]
                # slot indices (0..127 over branch-a's 4 I-blocks x 2 hp x 16 j)
                # at which to emit each segment: every 16 slots
                pass3_at = {16 * s + 8: pass3_work[s] for s in range(8)}

                otf_q = [[otfpool.tile([128, IB], bf16, name=f"otf{I}_{c}")
                          for c in range(4)] for I in range(NI)]

                battn = [(qTa, kTa), (qTp, kTp), (qTk, kTk)]

                def emit_proj_quarter(I):
                    """AllGather this I-block's attention output across the
                    core pair and apply the output projection."""
                    isl = slice(IB * I, IB * (I + 1))
                    for c in range(2):
                        nc.sync.dma_start(
                            cc_in_q[I][128 * c:128 * (c + 1), :], otc[c][:, isl])
                    nc.gpsimd.collective_compute(
                        "AllGather",
                        mybir.AluOpType.bypass,
                        replica_groups=[[0, 1], [2, 3], [4, 5], [6, 7]],
                        ins=[cc_in_q[I].opt()],
                        outs=[cc_out_q[I].opt()],
                    )
                    for c in range(4):
                        nc.sync.dma_start(
                            otf_q[I][c][:], cc_out_q[I][128 * c:128 * (c + 1), :])
                    for ot in range(4):
                        ps = aux_ps_pool.tile([128, IB], f32, tag="aux", name="fps")
                        for ic in range(4):
                            nc.tensor.matmul(
                                ps[:], wo_sb[ic][:, 128 * ot:128 * (ot + 1)],
                                otf_q[I][ic][:], start=(ic == 0), stop=(ic == 3))
                        fin = epool.tile([128, IB], bf16, tag="e", name="fin")
                        nc.vector.tensor_scalar_add(fin[:], ps[:],
                                                    bias_sb[:, ot:ot + 1])
                        nc.sync.dma_start(
                            out[128 * ot:128 * (ot + 1), isl], fin[:])

                slot = 0
                for bi, (qT_t, kT_t) in enumerate(battn):
                    d = (da, dp, dk)[bi]
                    for I in range(NI):
                        isl = slice(IB * I, IB * (I + 1))
                        for hp in range(2):
                            o_ps_h = [o_ps_pool.tile([128, IB], f32, tag="o",
                                                     name=f"ops{hh}")
                                      for hh in range(2)]

                            def emit_av(jj, pp_sb):
                                for hh in range(2):
                                    h = 2 * hp + hh
                                    nc.tensor.matmul(
                                        o_ps_h[hh][:],
                                        vaug[h][:, 128 * jj:128 * (jj + 1)],
                                        pp_sb[:, IB * hh:IB * (hh + 1)],
                                        start=(jj == 0), stop=(jj == NJ - 1),
                                        skip_group_check=True)

                            av_backlog = []
                            for j in range(NJ):
                                s_ps = s_ps_pool.tile([128, 2 * IB], f32,
                                                      tag="s", name=f"sh{hp}")
                                dots = []
                                for hh in range(2):
                                    h = 2 * hp + hh
                                    pb = 32 * h
                                    mm = nc.tensor.matmul(
                                        s_ps[:, IB * hh:IB * (hh + 1)],
                                        kT_t[pb:pb + d, 128 * j:128 * (j + 1)],
                                        qT_t[pb:pb + d, isl],
                                        start=True, stop=True,
                                        tile_position=(pb, 0))
                                    if dots:
                                        add_dep_helper(mm.ins, dots[-1].ins,
                                                       sync=False,
                                                       reason="chain dots")
                                    dots.append(mm)
                                while av_backlog and av_backlog[0][0] <= j:
                                    _, jj, pp_sb = av_backlog.pop(0)
                                    emit_av(jj, pp_sb)
                                e_sb = epool.tile([128, 2 * IB], bf16, tag="e")
                                nc.scalar.activation(e_sb[:], s_ps[:], Exp)
                                p_sb = ppool.tile([128, 2 * IB], bf16, tag="p")
                                m_bc = m_sb[j][:, None, isl].broadcast_to(
                                    [128, 2, IB])
                                # a chunk of the mask multiplies go to the
                                # idle gpsimd engine -- but only while no
                                # collective shares its queue (branches a, p)
                                # and never so late that a deferred AV would
                                # land after the jj==NJ-1 stop flag
                                on_gp = (bi < 2) and (j % 3 == 2) and (j < 12)
                                teng = nc.gpsimd if on_gp else nc.vector
                                teng.tensor_tensor(
                                    p_sb[:].rearrange("p (g i) -> p g i", g=2),
                                    e_sb[:].rearrange("p (g i) -> p g i", g=2),
                                    m_bc, op=mult)
                                if on_gp:
                                    av_backlog.append((j + 2, j, p_sb))
                                else:
                                    emit_av(j, p_sb)
                                if slot in pass3_at:
                                    pass3_segment(*pass3_at[slot])
                                slot += 1
                            for _, jj, pp_sb in av_backlog:
                                emit_av(jj, pp_sb)
                            # epilogue: drain both accumulators, then
                            # normalize + accumulate into oacc
                            o_sbs = []
                            for hh in range(2):
                                o_sb = opool.tile([65, IB], f32, tag="osb",
                                                  name=f"osb{hh}")
                                nc.vector.tensor_copy(o_sb[:], o_ps_h[hh][0:65, :])
                                o_sbs.append(o_sb)
                            for hh in range(2):
                                h = 2 * hp + hh
                                for s in range(IB // 128):
                                    tp = aux_ps_pool.tile([128, 65], f32, tag="aux",
                                                          name="tps")
                                    nc.tensor.transpose(
                                        tp[:], o_sbs[hh][:, 128 * s:128 * (s + 1)],
                                        ident_f32[0:65, 0:65])
                                    r_sb = rpool.tile([128, 1], f32, tag="r")
                                    nc.vector.reciprocal(r_sb[:], tp[:, 64:65])
                                    at = oacc[h][4 * I + s]
                                    if bi == 0:
                                        nc.vector.tensor_scalar_mul(at[:], tp[:, 0:DV], r_sb[:])
                                    else:
                                        nc.vector.scalar_tensor_tensor(
                                            at[:], tp[:, 0:DV], r_sb[:], at[:],
                                            op0=mult, op1=add)
                                    if bi == 2:
                                        tp2 = aux_ps_pool.tile([DV, 128], f32,
                                                               tag="aux", name="t2")
                                        nc.tensor.transpose(tp2[:], at[:],
                                                            ident_f32[:])
                                        sl = 4 * I + s
                                        nc.vector.tensor_copy(
                                            otc[h // 2][64 * (h % 2):64 * (h % 2) + DV,
                                                        128 * sl:128 * (sl + 1)],
                                            tp2[:])
                        if bi == 2:
                            # this I-block's attention output is final:
                            # gather + project it now so the collective and
                            # projection hide under the attention tail
                            emit_proj_quarter(I)

            _mctx.__exit__(None, None, None)

    nc.compile()
    return nc


def _prep_core(c, x, W_a, W_p, W_k, W_out, b_out, mask):
    b = c // 2
    h0 = H_LOC * (c % 2)

    xT = np.ascontiguousarray(x[b].T).astype(BF16)
    maskT = np.ascontiguousarray(mask[b, 0].T).astype(BF16)

    qa = W_a[da * h0: da * (h0 + H_LOC), :] * (DA ** -0.5)
    ka = W_a[DA_H + da * h0: DA_H + da * (h0 + H_LOC), :]
    va = W_a[2 * DA_H + da * h0: 2 * DA_H + da * (h0 + H_LOC), :]
    waT = np.concatenate([qa.T, ka.T, va.T], axis=1).astype(BF16)

    def pk_branch(W, D, D_H, d, vcol_ofs):
        qpad = np.zeros((D, 128), np.float32)
        kpad = np.zeros((D, 128), np.float32)
        vpad = np.zeros((D, 128), np.float32)
        for h in range(H_LOC):
            qpad[:, 32 * h:32 * h + d] = W[d * (h0 + h): d * (h0 + h + 1), :].T * (D ** -0.5)
            kpad[:, 32 * h:32 * h + d] = W[D_H + d * (h0 + h): D_H + d * (h0 + h + 1), :].T
            vpad[:, 32 * h + vcol_ofs:32 * h + vcol_ofs + d] = \
                W[2 * D_H + d * (h0 + h): 2 * D_H + d * (h0 + h + 1), :].T
        return np.concatenate([qpad, kpad, vpad], axis=1).astype(BF16)

    wpT = pk_branch(W_p, DP, DP_H, dp, 0)
    wkT = pk_branch(W_k, DK, DK_H, dk, 16)

    woutT = np.ascontiguousarray((W_out / 3.0).T).astype(BF16)
    bout = np.ascontiguousarray(b_out.reshape(DOUT, 1)).astype(np.float32)

    return {
        "xT": np.ascontiguousarray(xT),
        "maskT": np.ascontiguousarray(maskT),
        "waT": np.ascontiguousarray(waT),
        "wpT": np.ascontiguousarray(wpT),
        "wkT": np.ascontiguousarray(wkT),
        "woutT": woutT,
        "bout": bout,
    }


def kernel(x, W_a, W_p, W_k, W_out, b_out, mask):
    from concourse.bass_utils import run_bass_kernel_spmd

    x = np.asarray(x, np.float32)
    W_a = np.asarray(W_a, np.float32)
    W_p = np.asarray(W_p, np.float32)
    W_k = np.asarray(W_k, np.float32)
    W_out = np.asarray(W_out, np.float32)
    b_out = np.asarray(b_out, np.float32)
    mask = np.asarray(mask)

    if "nc" not in _CACHE:
        _CACHE["nc"] = _build()
    nc = _CACHE["nc"]

    in_maps = [_prep_core(c, x, W_a, W_p, W_k, W_out, b_out, mask)
               for c in range(NCORES)]
    res = run_bass_kernel_spmd(nc, in_maps, core_ids=list(range(NCORES)))

    outs = []
    for b in range(B):
        outs.append(np.asarray(res.results[2 * b]["out"]).astype(np.float32).T)
    return np.stack(outs, axis=0)


# revision 14
# speedup vs baseline: 1.2146x; 1.2146x over previous
"""Distributed Trainium2 kernel for the 3-branch masked attention problem.

Sharding: 8 cores; core c handles batch b = c//2 and heads h0 = 4*(c%2) .. +4
(data + head parallel).  Each core computes QKV for its heads, the three
branch softmaxes and AV locally; per-I-block (512-token) AllGathers of the
attention output let both cores of a batch apply the output projection with
the collectives overlapped under the attention tail.

Schedule (v3): the scalar engine's exp stream is the critical resource
(50.3M exps/core at 1 elem/cycle/lane = ~440us).  Everything else hides
under it: QKV is split so attention starts as soon as branch-a q/k and the
combined V are ready, the p/k q/k projections interleave into branch-a
attention's PE slack, a quarter of the mask multiplies go to the idle
gpsimd engine, and all block epilogues / output projections are emitted
through a deferred-work queue drained one item per j-iteration so nothing
ever head-of-line-blocks the dots->exp chain on the PE queue.
"""

import numpy as np
import ml_dtypes

BF16 = ml_dtypes.bfloat16

H = 8
DA, DP, DK = 2048, 1024, 1024
B, N = 4, 2048
DOUT = 512
H_LOC = 4           # heads per core
DA_H, DP_H, DK_H = DA // H, DP // H, DK // H      # 256, 128, 128
da, dp, dk = DA_H // H, DP_H // H, DK_H // H      # 32, 16, 16
DV = da + dp + dk                                 # 64
NCORES = 8

IB = 512            # query block (moving dim of dots / AV)
NI = N // IB        # 4
NJ = N // 128       # 16

_CACHE = {}


def _build():
    import concourse.bass as bass
    import concourse.mybir as mybir
    import concourse.tile as tile
    from concourse import bacc
    from concourse.masks import make_identity
    from concourse.tile import add_dep_helper

    f32 = mybir.dt.float32
    bf16 = mybir.dt.bfloat16
    Exp = mybir.ActivationFunctionType.Exp
    mult = mybir.AluOpType.mult
    add = mybir.AluOpType.add

    nc = bacc.Bacc("TRN2", target_bir_lowering=False, debug=False,
                   enable_asserts=False, num_devices=NCORES)

    xT = nc.dram_tensor("xT", [DA + DP + DK, N], bf16, kind="ExternalInput")
    maskT = nc.dram_tensor("maskT", [N, N], bf16, kind="ExternalInput")
    waT = nc.dram_tensor("waT", [DA, 384], bf16, kind="ExternalInput")
    wpT = nc.dram_tensor("wpT", [DP, 384], bf16, kind="ExternalInput")
    wkT = nc.dram_tensor("wkT", [DK, 384], bf16, kind="ExternalInput")
    woutT = nc.dram_tensor("woutT", [DOUT, DOUT], bf16, kind="ExternalInput")
    bout = nc.dram_tensor("bout", [DOUT, 1], f32, kind="ExternalInput")
    out = nc.dram_tensor("out", [DOUT, N], bf16, kind="ExternalOutput")

    with tile.TileContext(nc) as tc:
        with (
            tc.tile_pool(name="const", bufs=1) as cpool,
            tc.tile_pool(name="dram", bufs=1, space="DRAM") as dpool,
        ):
            # ---- constants (weights as bulk DMAs) ----
            ident_bf = cpool.tile([128, 128], bf16)
            make_identity(nc, ident_bf)
            ident_f32 = cpool.tile([128, 128], f32)
            make_identity(nc, ident_f32)

            wa_sb = [cpool.tile([128, 384], bf16, name=f"wa{f}") for f in range(16)]
            for f in range(16):
                nc.sync.dma_start(wa_sb[f][:], waT[128 * f:128 * (f + 1), :])

            bias_sb = cpool.tile([128, 4], f32)
            for t in range(4):
                nc.sync.dma_start(bias_sb[:, t:t + 1], bout[128 * t:128 * (t + 1), :])

            # ---- persistent activations ----
            qTa = cpool.tile([128, N], bf16)
            kTa = cpool.tile([128, N], bf16)
            qTp = cpool.tile([128, N], bf16)
            kTp = cpool.tile([128, N], bf16)
            qTk = cpool.tile([128, N], bf16)
            kTk = cpool.tile([128, N], bf16)
            # comb_va rows 32h+[0:32] = va of head h
            # comb_vpk rows 32h+[vp(16)|vk(16)]
            comb_va = cpool.tile([128, N], bf16, name="cva")
            comb_vpk = cpool.tile([128, N], bf16, name="cvpk")
            # V_aug per head: 16 chunks of [128, 128]: cols 0:64 = v,
            # col 64 = ones, 65:128 = zeros
            vaug = [cpool.tile([128, 128 * NJ], bf16, name=f"vaug{h}") for h in range(H_LOC)]
            # normalized attention output accumulator, [token, dv] layout
            oacc = [[cpool.tile([128, DV], f32, name=f"oacc{h}_{s}") for s in range(N // 128)]
                    for h in range(H_LOC)]
            # final transposed attention output (this core's heads)
            otc = [cpool.tile([128, N], bf16, name=f"otc{i}") for i in range(2)]

            # per-I-block collective bounce buffers (quarter gathers)
            cc_in_q = [dpool.tile([2 * 128, IB], bf16, name=f"ccin{I}")
                       for I in range(NI)]
            cc_out_q = [dpool.tile([4 * 128, IB], bf16, name=f"ccout{I}")
                        for I in range(NI)]

            for h in range(H_LOC):
                nc.gpsimd.memset(vaug[h][:], 0.0)
                for j in range(NJ):
                    nc.gpsimd.memset(vaug[h][:, 128 * j + 64:128 * j + 65], 1.0)

            # warm the exp table set off the critical path
            warm_sb = cpool.tile([128, 16], bf16)
            nc.scalar.activation(warm_sb[:], ident_f32[:, 0:16], Exp)

            _mctx = tc.tile_pool(name="mask", bufs=1)
            mpool = _mctx.__enter__()
            m_sb = [mpool.tile([128, N], bf16, name=f"m{j}") for j in range(NJ)]
            # first mask tiles up front so the attention start never races
            # the mask DMA
            for j in range(2):
                nc.sync.dma_start(m_sb[j][:], maskT[128 * j:128 * (j + 1), :])

            # =================== QKV projection ===================
            # pass1: branch a complete (q, k, v in one x stream)
            # pass2v: p/k v projections (shared accumulator, one x stream)
            # pass3 (emitted interleaved into branch-a attention): p/k q,k
            with (
                tc.tile_pool(name="xs", bufs=8) as xpool,
                tc.tile_pool(name="qkv_ps", bufs=6, space="PSUM") as qkv_ps,
            ):
                for tp2 in range(2):
                    t0 = 2 * IB * tp2
                    ps_q = [qkv_ps.tile([128, IB], f32, tag="qkv", name=f"psq{u}")
                            for u in range(2)]
                    ps_k = [qkv_ps.tile([128, IB], f32, tag="qkv", name=f"psk{u}")
                            for u in range(2)]
                    ps_va = [qkv_ps.tile([128, IB], f32, tag="qkv", name=f"psva{u}")
                             for u in range(2)]
                    for f in range(16):
                        xt = xpool.tile([128, 2 * IB], bf16, tag="x")
                        nc.sync.dma_start(
                            xt[:], xT[128 * f:128 * (f + 1), t0:t0 + 2 * IB])
                        st, sp = (f == 0), (f == 15)
                        w = wa_sb[f]
                        for u in range(2):
                            xu = xt[:, IB * u:IB * (u + 1)]
                            nc.tensor.matmul(ps_q[u][:], w[:, 0:128], xu, start=st, stop=sp)
                            nc.tensor.matmul(ps_k[u][:], w[:, 128:256], xu, start=st, stop=sp)
                            nc.tensor.matmul(ps_va[u][:], w[:, 256:384], xu, start=st, stop=sp)
                    if tp2 == 0:
                        # p/k/out weights ride the DMA queue behind the
                        # first half of the x stream
                        wp_sb = [cpool.tile([128, 384], bf16, name=f"wp{f}")
                                 for f in range(8)]
                        wk_sb = [cpool.tile([128, 384], bf16, name=f"wk{f}")
                                 for f in range(8)]
                        for f in range(8):
                            nc.sync.dma_start(wp_sb[f][:],
                                              wpT[128 * f:128 * (f + 1), :])
                            nc.sync.dma_start(wk_sb[f][:],
                                              wkT[128 * f:128 * (f + 1), :])
                        wo_sb = [cpool.tile([128, DOUT], bf16, name=f"wo{f}")
                                 for f in range(4)]
                        for f in range(4):
                            nc.sync.dma_start(wo_sb[f][:],
                                              woutT[128 * f:128 * (f + 1), :])
                    for u in range(2):
                        tsl = slice(t0 + IB * u, t0 + IB * (u + 1))
                        nc.vector.tensor_copy(qTa[:, tsl], ps_q[u][:])
                        nc.vector.tensor_copy(kTa[:, tsl], ps_k[u][:])
                        nc.vector.tensor_copy(comb_va[:, tsl], ps_va[u][:])

                # pass2v: p+k v projections (shared accumulator); remaining
                # mask tiles interleave into the x stream so they arrive in
                # j order slightly ahead of the exp stream's needs
                mask_next = 2
                for tp2 in range(2):
                    t0 = 2 * IB * tp2
                    ps_vpk = [qkv_ps.tile([128, IB], f32, tag="qkv", name=f"psvpk{u}")
                              for u in range(2)]
                    for bi, wsb in ((1, wp_sb), (2, wk_sb)):
                        fofs = DA if bi == 1 else DA + DP
                        for f in range(8):
                            xt = xpool.tile([128, 2 * IB], bf16, tag="x")
                            nc.sync.dma_start(
                                xt[:], xT[fofs + 128 * f:fofs + 128 * (f + 1),
                                          t0:t0 + 2 * IB])
                            if mask_next < NJ:
                                nc.sync.dma_start(
                                    m_sb[mask_next][:],
                                    maskT[128 * mask_next:128 * (mask_next + 1), :])
                                mask_next += 1
                            vst = (f == 0) and bi == 1
                            vsp = (f == 7) and bi == 2
                            for u in range(2):
                                nc.tensor.matmul(ps_vpk[u][:], wsb[f][:, 256:384],
                                                 xt[:, IB * u:IB * (u + 1)],
                                                 start=vst, stop=vsp)
                    for u in range(2):
                        tsl = slice(t0 + IB * u, t0 + IB * (u + 1))
                        nc.vector.tensor_copy(comb_vpk[:, tsl], ps_vpk[u][:])

            # =================== attention ===================
            with (
                tc.tile_pool(name="otf", bufs=8) as otfpool,
                tc.tile_pool(name="s_ps", bufs=2, space="PSUM") as s_ps_pool,
                tc.tile_pool(name="o_ps", bufs=2, space="PSUM") as o_ps_pool,
                tc.tile_pool(name="aux_ps", bufs=2, space="PSUM") as aux_ps_pool,
                tc.tile_pool(name="ep", bufs=4) as epool,
                tc.tile_pool(name="pp", bufs=5) as ppool,
                tc.tile_pool(name="ob", bufs=3) as opool,
                tc.tile_pool(name="rr", bufs=4) as rpool,
                tc.tile_pool(name="x3", bufs=2) as x3pool,
            ):
                # V_aug: transpose comb chunks; transposed cols 32h:32h+32
                # belong to head h
                for j in range(NJ):
                    jsl = slice(128 * j, 128 * (j + 1))
                    tpv = aux_ps_pool.tile([128, 128], bf16, tag="aux", name="tpv")
                    nc.tensor.transpose(tpv[:], comb_va[:, jsl], ident_bf[:])
                    for h in range(H_LOC):
                        nc.vector.tensor_copy(vaug[h][:, 128 * j:128 * j + 32],
                                              tpv[:, 32 * h:32 * h + 32])
                    tpw = aux_ps_pool.tile([128, 128], bf16, tag="aux", name="tpw")
                    nc.tensor.transpose(tpw[:], comb_vpk[:, jsl], ident_bf[:])
                    for h in range(H_LOC):
                        nc.vector.tensor_copy(vaug[h][:, 128 * j + 32:128 * j + 64],
                                              tpw[:, 32 * h:32 * h + 32])

                # -------- deferred-work queue --------
                deferred = []

                def drain_one():
                    if deferred:
                        deferred.pop(0)()

                def drain_all():
                    while deferred:
                        deferred.pop(0)()

                # pass3: p/k q,k projections, emitted in segments between
                # branch-a attention iterations (PE slack under the
                # ACT-bound exp stream)
                def pass3_segment(bi3, tp2, u):
                    wsb = wp_sb if bi3 == 1 else wk_sb
                    fofs = DA if bi3 == 1 else DA + DP
                    t0 = 2 * IB * tp2
                    usl = slice(t0 + IB * u, t0 + IB * (u + 1))
                    ps_q3 = aux_ps_pool.tile([128, IB], f32, tag="aux", name="psq3")
                    ps_k3 = aux_ps_pool.tile([128, IB], f32, tag="aux", name="psk3")
                    for f in range(8):
                        xt = x3pool.tile([128, IB], bf16, tag="x3")
                        nc.sync.dma_start(
                            xt[:], xT[fofs + 128 * f:fofs + 128 * (f + 1), usl])
                        st, sp = (f == 0), (f == 7)
                        nc.tensor.matmul(ps_q3[:], wsb[f][:, 0:128], xt[:],
                                         start=st, stop=sp)
                        nc.tensor.matmul(ps_k3[:], wsb[f][:, 128:256], xt[:],
                                         start=st, stop=sp)
                    qT3 = qTp if bi3 == 1 else qTk
                    kT3 = kTp if bi3 == 1 else kTk
                    nc.vector.tensor_copy(qT3[:, usl], ps_q3[:])
                    nc.vector.tensor_copy(kT3[:, usl], ps_k3[:])

                pass3_work = [(bi3, tp2, u) for bi3 in (1, 2)
                              for tp2 in range(2) for u in range(2)]
                pass3_at = {16 * s + 4: pass3_work[s] for s in range(8)}

                battn = [(qTa, kTa), (qTp, kTp), (qTk, kTk)]

                def make_epilogue(bi, I, hp, o_ps_h):
                    """Split the block epilogue into small deferred items."""
                    o_sbs = [None, None]

                    def drain_o(hh):
                        def fn():
                            o_sb = opool.tile([65, IB], f32, tag="osb",
                                              name=f"osb{hh}")
                            nc.vector.tensor_copy(o_sb[:], o_ps_h[hh][0:65, :])
                            o_sbs[hh] = o_sb
                        return fn

                    def norm(hh, s):
                        def fn():
                            h = 2 * hp + hh
                            tp = aux_ps_pool.tile([128, 65], f32, tag="aux",
                                                  name="tps")
                            nc.tensor.transpose(
                                tp[:], o_sbs[hh][:, 128 * s:128 * (s + 1)],
                                ident_f32[0:65, 0:65])
                            r_sb = rpool.tile([128, 1], f32, tag="r")
                            nc.vector.reciprocal(r_sb[:], tp[:, 64:65])
                            at = oacc[h][4 * I + s]
                            if bi == 0:
                                nc.vector.tensor_scalar_mul(at[:], tp[:, 0:DV], r_sb[:])
                            else:
                                nc.vector.scalar_tensor_tensor(
                                    at[:], tp[:, 0:DV], r_sb[:], at[:],
                                    op0=mult, op1=add)
                            if bi == 2:
                                tp2t = aux_ps_pool.tile([DV, 128], f32,
                                                        tag="aux", name="t2")
                                nc.tensor.transpose(tp2t[:], at[:], ident_f32[:])
                                sl = 4 * I + s
                                nc.vector.tensor_copy(
                                    otc[h // 2][64 * (h % 2):64 * (h % 2) + DV,
                                                128 * sl:128 * (sl + 1)],
                                    tp2t[:])
                        return fn

                    items = [drain_o(0), drain_o(1)]
                    for hh in range(2):
                        for s in range(IB // 128):
                            items.append(norm(hh, s))
                    return items

                def emit_gather(I):
                    """Collective + otf loads for quarter I (sync/gpsimd/CC
                    queues only -- no PE work)."""
                    isl = slice(IB * I, IB * (I + 1))
                    for c in range(2):
                        nc.sync.dma_start(
                            cc_in_q[I][128 * c:128 * (c + 1), :], otc[c][:, isl])
                    nc.gpsimd.collective_compute(
                        "AllGather", mybir.AluOpType.bypass,
                        replica_groups=[[0, 1], [2, 3], [4, 5], [6, 7]],
                        ins=[cc_in_q[I].opt()], outs=[cc_out_q[I].opt()])
                    otf = []
                    for c in range(4):
                        t = otfpool.tile([128, IB], bf16, tag="otf",
                                         name=f"otf{c}")
                        nc.sync.dma_start(t[:], cc_out_q[I][128 * c:128 * (c + 1), :])
                        otf.append(t)
                    return otf

                def make_proj(I, otf):
                    def proj_ot(ot):
                        def fn():
                            isl = slice(IB * I, IB * (I + 1))
                            ps = aux_ps_pool.tile([128, IB], f32, tag="aux",
                                                  name="fps")
                            for ic in range(4):
                                nc.tensor.matmul(
                                    ps[:], wo_sb[ic][:, 128 * ot:128 * (ot + 1)],
                                    otf[ic][:], start=(ic == 0), stop=(ic == 3))
                            fin = epool.tile([128, IB], bf16, tag="e", name="fin")
                            nc.vector.tensor_scalar_add(fin[:], ps[:],
                                                        bias_sb[:, ot:ot + 1])
                            nc.sync.dma_start(
                                out[128 * ot:128 * (ot + 1), isl], fin[:])
                        return fn
                    return [proj_ot(ot) for ot in range(4)]

                slot = 0
                for bi, (qT_t, kT_t) in enumerate(battn):
                    d = (da, dp, dk)[bi]
                    for I in range(NI):
                        isl = slice(IB * I, IB * (I + 1))
                        for hp in range(2):
                            o_ps_h = [o_ps_pool.tile([128, IB], f32, tag="o",
                                                     name=f"ops{hh}")
                                      for hh in range(2)]

                            def emit_av(jj, pp_sb):
                                for hh in range(2):
                                    h = 2 * hp + hh
                                    nc.tensor.matmul(
                                        o_ps_h[hh][:],
                                        vaug[h][:, 128 * jj:128 * (jj + 1)],
                                        pp_sb[:, IB * hh:IB * (hh + 1)],
                                        start=(jj == 0), stop=(jj == NJ - 1),
                                        skip_group_check=True)

                            av_backlog = []
                            for j in range(NJ):
                                s_ps = s_ps_pool.tile([128, 2 * IB], f32,
                                                      tag="s", name=f"sh{hp}")
                                dots = []
                                for hh in range(2):
                                    h = 2 * hp + hh
                                    pb = 32 * h
                                    mm = nc.tensor.matmul(
                                        s_ps[:, IB * hh:IB * (hh + 1)],
                                        kT_t[pb:pb + d, 128 * j:128 * (j + 1)],
                                        qT_t[pb:pb + d, isl],
                                        start=True, stop=True,
                                        tile_position=(pb, 0))
                                    if dots:
                                        add_dep_helper(mm.ins, dots[-1].ins,
                                                       sync=False,
                                                       reason="chain dots")
                                    dots.append(mm)
                                while av_backlog and av_backlog[0][0] <= j:
                                    _, jj, pp_sb = av_backlog.pop(0)
                                    emit_av(jj, pp_sb)
                                e_sb = epool.tile([128, 2 * IB], bf16, tag="e")
                                nc.scalar.activation(e_sb[:], s_ps[:], Exp)
                                p_sb = ppool.tile([128, 2 * IB], bf16, tag="p")
                                m_bc = m_sb[j][:, None, isl].broadcast_to(
                                    [128, 2, IB])
                                # some mask multiplies go to the idle gpsimd
                                # engine -- but only while no collective
                                # shares its queue (branches a, p), and never
                                # so late that a deferred AV would land after
                                # the jj==NJ-1 stop flag
                                on_gp = (bi < 2) and (j % 3 == 2) and (j < 12)
                                teng = nc.gpsimd if on_gp else nc.vector
                                teng.tensor_tensor(
                                    p_sb[:].rearrange("p (g i) -> p g i", g=2),
                                    e_sb[:].rearrange("p (g i) -> p g i", g=2),
                                    m_bc, op=mult)
                                if on_gp:
                                    av_backlog.append((j + 2, j, p_sb))
                                else:
                                    emit_av(j, p_sb)
                                if slot in pass3_at:
                                    pass3_segment(*pass3_at[slot])
                                else:
                                    drain_one()
                                slot += 1
                            for _, jj, pp_sb in av_backlog:
                                emit_av(jj, pp_sb)
                            deferred.extend(make_epilogue(bi, I, hp, o_ps_h))
                        if bi == 2:
                            # quarter I's attention output is complete once
                            # its epilogue items drain; queue the gather +
                            # projection behind them
                            def queue_gather(I=I):
                                otf = emit_gather(I)
                                deferred.extend(make_proj(I, otf))
                            deferred.append(queue_gather)
                drain_all()

            _mctx.__exit__(None, None, None)

    nc.compile()
    return nc


def _prep_core(c, x, W_a, W_p, W_k, W_out, b_out, mask):
    b = c // 2
    h0 = H_LOC * (c % 2)

    xT = np.ascontiguousarray(x[b].T).astype(BF16)
    maskT = np.ascontiguousarray(mask[b, 0].T).astype(BF16)

    qa = W_a[da * h0: da * (h0 + H_LOC), :] * (DA ** -0.5)
    ka = W_a[DA_H + da * h0: DA_H + da * (h0 + H_LOC), :]
    va = W_a[2 * DA_H + da * h0: 2 * DA_H + da * (h0 + H_LOC), :]
    waT = np.concatenate([qa.T, ka.T, va.T], axis=1).astype(BF16)

    def pk_branch(W, D, D_H, d, vcol_ofs):
        qpad = np.zeros((D, 128), np.float32)
        kpad = np.zeros((D, 128), np.float32)
        vpad = np.zeros((D, 128), np.float32)
        for h in range(H_LOC):
            qpad[:, 32 * h:32 * h + d] = W[d * (h0 + h): d * (h0 + h + 1), :].T * (D ** -0.5)
            kpad[:, 32 * h:32 * h + d] = W[D_H + d * (h0 + h): D_H + d * (h0 + h + 1), :].T
            vpad[:, 32 * h + vcol_ofs:32 * h + vcol_ofs + d] = \
                W[2 * D_H + d * (h0 + h): 2 * D_H + d * (h0 + h + 1), :].T
        return np.concatenate([qpad, kpad, vpad], axis=1).astype(BF16)

    wpT = pk_branch(W_p, DP, DP_H, dp, 0)
    wkT = pk_branch(W_k, DK, DK_H, dk, 16)

    woutT = np.ascontiguousarray((W_out / 3.0).T).astype(BF16)
    bout = np.ascontiguousarray(b_out.reshape(DOUT, 1)).astype(np.float32)

    return {
        "xT": np.ascontiguousarray(xT),
        "maskT": np.ascontiguousarray(maskT),
        "waT": np.ascontiguousarray(waT),
        "wpT": np.ascontiguousarray(wpT),
        "wkT": np.ascontiguousarray(wkT),
        "woutT": woutT,
        "bout": bout,
    }


def kernel(x, W_a, W_p, W_k, W_out, b_out, mask):
    from concourse.bass_utils import run_bass_kernel_spmd

    x = np.asarray(x, np.float32)
    W_a = np.asarray(W_a, np.float32)
    W_p = np.asarray(W_p, np.float32)
    W_k = np.asarray(W_k, np.float32)
    W_out = np.asarray(W_out, np.float32)
    b_out = np.asarray(b_out, np.float32)
    mask = np.asarray(mask)

    if "nc" not in _CACHE:
        _CACHE["nc"] = _build()
    nc = _CACHE["nc"]

    in_maps = [_prep_core(c, x, W_a, W_p, W_k, W_out, b_out, mask)
               for c in range(NCORES)]
    res = run_bass_kernel_spmd(nc, in_maps, core_ids=list(range(NCORES)))

    outs = []
    for b in range(B):
        outs.append(np.asarray(res.results[2 * b]["out"]).astype(np.float32).T)
    return np.stack(outs, axis=0)


# revision 15
# speedup vs baseline: 1.2431x; 1.0234x over previous
"""Distributed Trainium2 kernel for the 3-branch masked attention problem.

Sharding: 8 cores; core c handles batch b = c//2 and heads h0 = 4*(c%2) .. +4
(data + head parallel).  Each core computes QKV for its heads, the three
branch softmaxes and AV locally; per-I-block (512-token) AllGathers of the
attention output let both cores of a batch apply the output projection with
the collectives overlapped under the attention tail.

Schedule (v3): the scalar engine's exp stream is the critical resource
(50.3M exps/core at 1 elem/cycle/lane = ~440us).  Everything else hides
under it: QKV is split so attention starts as soon as branch-a q/k and the
combined V are ready, the p/k q/k projections interleave into branch-a
attention's PE slack, a quarter of the mask multiplies go to the idle
gpsimd engine, and all block epilogues / output projections are emitted
through a deferred-work queue drained one item per j-iteration so nothing
ever head-of-line-blocks the dots->exp chain on the PE queue.
"""

import numpy as np
import ml_dtypes

BF16 = ml_dtypes.bfloat16

H = 8
DA, DP, DK = 2048, 1024, 1024
B, N = 4, 2048
DOUT = 512
H_LOC = 4           # heads per core
DA_H, DP_H, DK_H = DA // H, DP // H, DK // H      # 256, 128, 128
da, dp, dk = DA_H // H, DP_H // H, DK_H // H      # 32, 16, 16
DV = da + dp + dk                                 # 64
NCORES = 8

IB = 512            # query block (moving dim of dots / AV)
NI = N // IB        # 4
NJ = N // 128       # 16

_CACHE = {}


def _build():
    import concourse.bass as bass
    import concourse.mybir as mybir
    import concourse.tile as tile
    from concourse import bacc
    from concourse.masks import make_identity
    from concourse.tile import add_dep_helper

    f32 = mybir.dt.float32
    bf16 = mybir.dt.bfloat16
    Exp = mybir.ActivationFunctionType.Exp
    mult = mybir.AluOpType.mult
    add = mybir.AluOpType.add

    nc = bacc.Bacc("TRN2", target_bir_lowering=False, debug=False,
                   enable_asserts=False, num_devices=NCORES)

    xT = nc.dram_tensor("xT", [DA + DP + DK, N], bf16, kind="ExternalInput")
    maskT = nc.dram_tensor("maskT", [N, N], bf16, kind="ExternalInput")
    waT = nc.dram_tensor("waT", [DA, 384], bf16, kind="ExternalInput")
    wpT = nc.dram_tensor("wpT", [DP, 384], bf16, kind="ExternalInput")
    wkT = nc.dram_tensor("wkT", [DK, 384], bf16, kind="ExternalInput")
    woutT = nc.dram_tensor("woutT", [DOUT, DOUT], bf16, kind="ExternalInput")
    bout = nc.dram_tensor("bout", [DOUT, 1], f32, kind="ExternalInput")
    out = nc.dram_tensor("out", [DOUT, N], bf16, kind="ExternalOutput")

    with tile.TileContext(nc) as tc:
        with (
            tc.tile_pool(name="const", bufs=1) as cpool,
            tc.tile_pool(name="dram", bufs=1, space="DRAM") as dpool,
        ):
            # ---- constants (weights as bulk DMAs) ----
            ident_bf = cpool.tile([128, 128], bf16)
            make_identity(nc, ident_bf)
            ident_f32 = cpool.tile([128, 128], f32)
            make_identity(nc, ident_f32)

            wa_sb = [cpool.tile([128, 384], bf16, name=f"wa{f}") for f in range(16)]
            for f in range(16):
                nc.sync.dma_start(wa_sb[f][:], waT[128 * f:128 * (f + 1), :])

            bias_sb = cpool.tile([128, 4], f32)
            for t in range(4):
                nc.sync.dma_start(bias_sb[:, t:t + 1], bout[128 * t:128 * (t + 1), :])

            # ---- persistent activations ----
            qTa = cpool.tile([128, N], bf16)
            kTa = cpool.tile([128, N], bf16)
            qTp = cpool.tile([128, N], bf16)
            kTp = cpool.tile([128, N], bf16)
            qTk = cpool.tile([128, N], bf16)
            kTk = cpool.tile([128, N], bf16)
            # comb_va rows 32h+[0:32] = va of head h
            # comb_vpk rows 32h+[vp(16)|vk(16)]
            comb_va = cpool.tile([128, N], bf16, name="cva")
            comb_vpk = cpool.tile([128, N], bf16, name="cvpk")
            # V_aug per head: 16 chunks of [128, 128]: cols 0:64 = v,
            # col 64 = ones, 65:128 = zeros
            vaug = [cpool.tile([128, 128 * NJ], bf16, name=f"vaug{h}") for h in range(H_LOC)]
            # normalized attention output accumulator, [token, dv] layout
            oacc = [[cpool.tile([128, DV], f32, name=f"oacc{h}_{s}") for s in range(N // 128)]
                    for h in range(H_LOC)]
            # final transposed attention output (this core's heads)
            otc = [cpool.tile([128, N], bf16, name=f"otc{i}") for i in range(2)]

            # per-I-block collective bounce buffers (quarter gathers)
            cc_in_q = [dpool.tile([2 * 128, IB], bf16, name=f"ccin{I}")
                       for I in range(NI)]
            cc_out_q = [dpool.tile([4 * 128, IB], bf16, name=f"ccout{I}")
                        for I in range(NI)]

            for h in range(H_LOC):
                nc.gpsimd.memset(vaug[h][:], 0.0)
                for j in range(NJ):
                    nc.gpsimd.memset(vaug[h][:, 128 * j + 64:128 * j + 65], 1.0)

            # warm the exp table set off the critical path
            warm_sb = cpool.tile([128, 16], bf16)
            nc.scalar.activation(warm_sb[:], ident_f32[:, 0:16], Exp)

            _mctx = tc.tile_pool(name="mask", bufs=1)
            mpool = _mctx.__enter__()
            m_sb = [mpool.tile([128, N], bf16, name=f"m{j}") for j in range(NJ)]
            # first mask tiles up front so the attention start never races
            # the mask DMA
            for j in range(2):
                nc.sync.dma_start(m_sb[j][:], maskT[128 * j:128 * (j + 1), :])

            # =================== QKV projection ===================
            # pass1: branch a complete (q, k, v in one x stream)
            # pass2v: p/k v projections (shared accumulator, one x stream)
            # pass3 (emitted interleaved into branch-a attention): p/k q,k
            with (
                tc.tile_pool(name="xs", bufs=8) as xpool,
                tc.tile_pool(name="qkv_ps", bufs=6, space="PSUM") as qkv_ps,
            ):
                for tp2 in range(2):
                    t0 = 2 * IB * tp2
                    ps_q = [qkv_ps.tile([128, IB], f32, tag="qkv", name=f"psq{u}")
                            for u in range(2)]
                    ps_k = [qkv_ps.tile([128, IB], f32, tag="qkv", name=f"psk{u}")
                            for u in range(2)]
                    ps_va = [qkv_ps.tile([128, IB], f32, tag="qkv", name=f"psva{u}")
                             for u in range(2)]
                    for f in range(16):
                        xt = xpool.tile([128, 2 * IB], bf16, tag="x")
                        nc.sync.dma_start(
                            xt[:], xT[128 * f:128 * (f + 1), t0:t0 + 2 * IB])
                        st, sp = (f == 0), (f == 15)
                        w = wa_sb[f]
                        for u in range(2):
                            xu = xt[:, IB * u:IB * (u + 1)]
                            nc.tensor.matmul(ps_q[u][:], w[:, 0:128], xu, start=st, stop=sp)
                            nc.tensor.matmul(ps_k[u][:], w[:, 128:256], xu, start=st, stop=sp)
                            nc.tensor.matmul(ps_va[u][:], w[:, 256:384], xu, start=st, stop=sp)
                    if tp2 == 0:
                        # p/k/out weights ride the DMA queue behind the
                        # first half of the x stream
                        wp_sb = [cpool.tile([128, 384], bf16, name=f"wp{f}")
                                 for f in range(8)]
                        wk_sb = [cpool.tile([128, 384], bf16, name=f"wk{f}")
                                 for f in range(8)]
                        for f in range(8):
                            nc.sync.dma_start(wp_sb[f][:],
                                              wpT[128 * f:128 * (f + 1), :])
                            nc.sync.dma_start(wk_sb[f][:],
                                              wkT[128 * f:128 * (f + 1), :])
                        wo_sb = [cpool.tile([128, DOUT], bf16, name=f"wo{f}")
                                 for f in range(4)]
                        for f in range(4):
                            nc.sync.dma_start(wo_sb[f][:],
                                              woutT[128 * f:128 * (f + 1), :])
                    for u in range(2):
                        tsl = slice(t0 + IB * u, t0 + IB * (u + 1))
                        nc.vector.tensor_copy(qTa[:, tsl], ps_q[u][:])
                        nc.vector.tensor_copy(kTa[:, tsl], ps_k[u][:])
                        nc.vector.tensor_copy(comb_va[:, tsl], ps_va[u][:])

                # pass2v: p+k v projections (shared accumulator); remaining
                # mask tiles interleave into the x stream so they arrive in
                # j order slightly ahead of the exp stream's needs
                mask_next = 2
                for tp2 in range(2):
                    t0 = 2 * IB * tp2
                    ps_vpk = [qkv_ps.tile([128, IB], f32, tag="qkv", name=f"psvpk{u}")
                              for u in range(2)]
                    for bi, wsb in ((1, wp_sb), (2, wk_sb)):
                        fofs = DA if bi == 1 else DA + DP
                        for f in range(8):
                            xt = xpool.tile([128, 2 * IB], bf16, tag="x")
                            nc.sync.dma_start(
                                xt[:], xT[fofs + 128 * f:fofs + 128 * (f + 1),
                                          t0:t0 + 2 * IB])
                            if mask_next < NJ:
                                nc.sync.dma_start(
                                    m_sb[mask_next][:],
                                    maskT[128 * mask_next:128 * (mask_next + 1), :])
                                mask_next += 1
                            vst = (f == 0) and bi == 1
                            vsp = (f == 7) and bi == 2
                            for u in range(2):
                                nc.tensor.matmul(ps_vpk[u][:], wsb[f][:, 256:384],
                                                 xt[:, IB * u:IB * (u + 1)],
                                                 start=vst, stop=vsp)
                    for u in range(2):
                        tsl = slice(t0 + IB * u, t0 + IB * (u + 1))
                        nc.vector.tensor_copy(comb_vpk[:, tsl], ps_vpk[u][:])

            # =================== attention ===================
            with (
                tc.tile_pool(name="otf", bufs=8) as otfpool,
                tc.tile_pool(name="s_ps", bufs=2, space="PSUM") as s_ps_pool,
                tc.tile_pool(name="o_ps", bufs=2, space="PSUM") as o_ps_pool,
                tc.tile_pool(name="aux_ps", bufs=2, space="PSUM") as aux_ps_pool,
                tc.tile_pool(name="ep", bufs=4) as epool,
                tc.tile_pool(name="pp", bufs=5) as ppool,
                tc.tile_pool(name="ob", bufs=3) as opool,
                tc.tile_pool(name="rr", bufs=4) as rpool,
                tc.tile_pool(name="x3", bufs=2) as x3pool,
                tc.tile_pool(name="fp", bufs=2) as fpool,
            ):
                # V_aug: transpose comb chunks; transposed cols 32h:32h+32
                # belong to head h
                for j in range(NJ):
                    jsl = slice(128 * j, 128 * (j + 1))
                    tpv = aux_ps_pool.tile([128, 128], bf16, tag="aux", name="tpv")
                    nc.tensor.transpose(tpv[:], comb_va[:, jsl], ident_bf[:])
                    for h in range(H_LOC):
                        nc.vector.tensor_copy(vaug[h][:, 128 * j:128 * j + 32],
                                              tpv[:, 32 * h:32 * h + 32])
                    tpw = aux_ps_pool.tile([128, 128], bf16, tag="aux", name="tpw")
                    nc.tensor.transpose(tpw[:], comb_vpk[:, jsl], ident_bf[:])
                    for h in range(H_LOC):
                        nc.vector.tensor_copy(vaug[h][:, 128 * j + 32:128 * j + 64],
                                              tpw[:, 32 * h:32 * h + 32])

                # -------- deferred-work queue --------
                deferred = []

                def drain_one():
                    if deferred:
                        deferred.pop(0)()

                def drain_all():
                    while deferred:
                        deferred.pop(0)()

                # pass3: p/k q,k projections, emitted in segments between
                # branch-a attention iterations (PE slack under the
                # ACT-bound exp stream)
                def pass3_segment(bi3, tp2, u):
                    wsb = wp_sb if bi3 == 1 else wk_sb
                    fofs = DA if bi3 == 1 else DA + DP
                    t0 = 2 * IB * tp2
                    usl = slice(t0 + IB * u, t0 + IB * (u + 1))
                    ps_q3 = aux_ps_pool.tile([128, IB], f32, tag="aux", name="psq3")
                    ps_k3 = aux_ps_pool.tile([128, IB], f32, tag="aux", name="psk3")
                    for f in range(8):
                        xt = x3pool.tile([128, IB], bf16, tag="x3")
                        nc.sync.dma_start(
                            xt[:], xT[fofs + 128 * f:fofs + 128 * (f + 1), usl])
                        st, sp = (f == 0), (f == 7)
                        nc.tensor.matmul(ps_q3[:], wsb[f][:, 0:128], xt[:],
                                         start=st, stop=sp)
                        nc.tensor.matmul(ps_k3[:], wsb[f][:, 128:256], xt[:],
                                         start=st, stop=sp)
                    qT3 = qTp if bi3 == 1 else qTk
                    kT3 = kTp if bi3 == 1 else kTk
                    nc.vector.tensor_copy(qT3[:, usl], ps_q3[:])
                    nc.vector.tensor_copy(kT3[:, usl], ps_k3[:])

                pass3_work = [(bi3, tp2, u) for bi3 in (1, 2)
                              for tp2 in range(2) for u in range(2)]
                pass3_at = {16 * s + 4: pass3_work[s] for s in range(8)}

                battn = [(qTa, kTa), (qTp, kTp), (qTk, kTk)]

                def make_epilogue(bi, I, hp, o_ps_h):
                    """Drain the accumulators eagerly (frees PSUM before the
                    next block's first AV); defer the normalize items."""
                    o_sbs = [None, None]
                    for hh in range(2):
                        o_sb = opool.tile([65, IB], f32, tag="osb",
                                          name=f"osb{hh}")
                        nc.vector.tensor_copy(o_sb[:], o_ps_h[hh][0:65, :])
                        o_sbs[hh] = o_sb

                    def norm(hh, s):
                        def fn():
                            h = 2 * hp + hh
                            tp = aux_ps_pool.tile([128, 65], f32, tag="aux",
                                                  name="tps")
                            nc.tensor.transpose(
                                tp[:], o_sbs[hh][:, 128 * s:128 * (s + 1)],
                                ident_f32[0:65, 0:65])
                            r_sb = rpool.tile([128, 1], f32, tag="r")
                            nc.vector.reciprocal(r_sb[:], tp[:, 64:65])
                            at = oacc[h][4 * I + s]
                            if bi == 0:
                                nc.vector.tensor_scalar_mul(at[:], tp[:, 0:DV], r_sb[:])
                            else:
                                nc.vector.scalar_tensor_tensor(
                                    at[:], tp[:, 0:DV], r_sb[:], at[:],
                                    op0=mult, op1=add)
                            if bi == 2:
                                tp2t = aux_ps_pool.tile([DV, 128], f32,
                                                        tag="aux", name="t2")
                                nc.tensor.transpose(tp2t[:], at[:], ident_f32[:])
                                sl = 4 * I + s
                                nc.vector.tensor_copy(
                                    otc[h // 2][64 * (h % 2):64 * (h % 2) + DV,
                                                128 * sl:128 * (sl + 1)],
                                    tp2t[:])
                        return fn

                    items = []
                    for hh in range(2):
                        for s in range(IB // 128):
                            items.append(norm(hh, s))
                    return items

                def emit_gather(I):
                    """Collective + otf loads for quarter I (sync/gpsimd/CC
                    queues only -- no PE work)."""
                    isl = slice(IB * I, IB * (I + 1))
                    for c in range(2):
                        nc.sync.dma_start(
                            cc_in_q[I][128 * c:128 * (c + 1), :], otc[c][:, isl])
                    nc.gpsimd.collective_compute(
                        "AllGather", mybir.AluOpType.bypass,
                        replica_groups=[[0, 1], [2, 3], [4, 5], [6, 7]],
                        ins=[cc_in_q[I].opt()], outs=[cc_out_q[I].opt()])
                    otf = []
                    for c in range(4):
                        t = otfpool.tile([128, IB], bf16, tag="otf",
                                         name=f"otf{c}")
                        nc.sync.dma_start(t[:], cc_out_q[I][128 * c:128 * (c + 1), :])
                        otf.append(t)
                    return otf

                def make_proj(I, otf):
                    def proj_ot(ot):
                        def fn():
                            isl = slice(IB * I, IB * (I + 1))
                            ps = aux_ps_pool.tile([128, IB], f32, tag="aux",
                                                  name="fps")
                            for ic in range(4):
                                nc.tensor.matmul(
                                    ps[:], wo_sb[ic][:, 128 * ot:128 * (ot + 1)],
                                    otf[ic][:], start=(ic == 0), stop=(ic == 3))
                            fin = fpool.tile([128, IB], bf16, tag="fin", name="fin")
                            nc.vector.tensor_scalar_add(fin[:], ps[:],
                                                        bias_sb[:, ot:ot + 1])
                            nc.sync.dma_start(
                                out[128 * ot:128 * (ot + 1), isl], fin[:])
                        return fn
                    return [proj_ot(ot) for ot in range(4)]

                slot = 0
                for bi, (qT_t, kT_t) in enumerate(battn):
                    d = (da, dp, dk)[bi]
                    for I in range(NI):
                        isl = slice(IB * I, IB * (I + 1))
                        for hp in range(2):
                            o_ps_h = [o_ps_pool.tile([128, IB], f32, tag="o",
                                                     name=f"ops{hh}")
                                      for hh in range(2)]

                            def emit_av(jj, pp_sb):
                                for hh in range(2):
                                    h = 2 * hp + hh
                                    nc.tensor.matmul(
                                        o_ps_h[hh][:],
                                        vaug[h][:, 128 * jj:128 * (jj + 1)],
                                        pp_sb[:, IB * hh:IB * (hh + 1)],
                                        start=(jj == 0), stop=(jj == NJ - 1),
                                        skip_group_check=True)

                            av_backlog = []
                            for j in range(NJ):
                                s_ps = s_ps_pool.tile([128, 2 * IB], f32,
                                                      tag="s", name=f"sh{hp}")
                                dots = []
                                for hh in range(2):
                                    h = 2 * hp + hh
                                    pb = 32 * h
                                    mm = nc.tensor.matmul(
                                        s_ps[:, IB * hh:IB * (hh + 1)],
                                        kT_t[pb:pb + d, 128 * j:128 * (j + 1)],
                                        qT_t[pb:pb + d, isl],
                                        start=True, stop=True,
                                        tile_position=(pb, 0))
                                    if dots:
                                        add_dep_helper(mm.ins, dots[-1].ins,
                                                       sync=False,
                                                       reason="chain dots")
                                    dots.append(mm)
                                while av_backlog and av_backlog[0][0] <= j:
                                    _, jj, pp_sb = av_backlog.pop(0)
                                    emit_av(jj, pp_sb)
                                e_sb = epool.tile([128, 2 * IB], bf16, tag="e")
                                nc.scalar.activation(e_sb[:], s_ps[:], Exp)
                                p_sb = ppool.tile([128, 2 * IB], bf16, tag="p")
                                m_bc = m_sb[j][:, None, isl].broadcast_to(
                                    [128, 2, IB])
                                # some mask multiplies go to the idle gpsimd
                                # engine -- but only while no collective
                                # shares its queue (branches a, p), and never
                                # so late that a deferred AV would land after
                                # the jj==NJ-1 stop flag
                                on_gp = (bi < 2) and (j % 3 == 2) and (j < 12)
                                teng = nc.gpsimd if on_gp else nc.vector
                                teng.tensor_tensor(
                                    p_sb[:].rearrange("p (g i) -> p g i", g=2),
                                    e_sb[:].rearrange("p (g i) -> p g i", g=2),
                                    m_bc, op=mult)
                                if on_gp:
                                    av_backlog.append((j + 2, j, p_sb))
                                else:
                                    emit_av(j, p_sb)
                                if slot in pass3_at:
                                    pass3_segment(*pass3_at[slot])
                                else:
                                    drain_one()
                                slot += 1
                            for _, jj, pp_sb in av_backlog:
                                emit_av(jj, pp_sb)
                            deferred.extend(make_epilogue(bi, I, hp, o_ps_h))
                        if bi == 2:
                            # quarter I's attention output is complete once
                            # its epilogue items drain; queue the gather +
                            # projection behind them
                            def queue_gather(I=I):
                                otf = emit_gather(I)
                                deferred.extend(make_proj(I, otf))
                            deferred.append(queue_gather)
                drain_all()

            _mctx.__exit__(None, None, None)

    nc.compile()
    return nc


def _prep_core(c, x, W_a, W_p, W_k, W_out, b_out, mask):
    b = c // 2
    h0 = H_LOC * (c % 2)

    xT = np.ascontiguousarray(x[b].T).astype(BF16)
    maskT = np.ascontiguousarray(mask[b, 0].T).astype(BF16)

    qa = W_a[da * h0: da * (h0 + H_LOC), :] * (DA ** -0.5)
    ka = W_a[DA_H + da * h0: DA_H + da * (h0 + H_LOC), :]
    va = W_a[2 * DA_H + da * h0: 2 * DA_H + da * (h0 + H_LOC), :]
    waT = np.concatenate([qa.T, ka.T, va.T], axis=1).astype(BF16)

    def pk_branch(W, D, D_H, d, vcol_ofs):
        qpad = np.zeros((D, 128), np.float32)
        kpad = np.zeros((D, 128), np.float32)
        vpad = np.zeros((D, 128), np.float32)
        for h in range(H_LOC):
            qpad[:, 32 * h:32 * h + d] = W[d * (h0 + h): d * (h0 + h + 1), :].T * (D ** -0.5)
            kpad[:, 32 * h:32 * h + d] = W[D_H + d * (h0 + h): D_H + d * (h0 + h + 1), :].T
            vpad[:, 32 * h + vcol_ofs:32 * h + vcol_ofs + d] = \
                W[2 * D_H + d * (h0 + h): 2 * D_H + d * (h0 + h + 1), :].T
        return np.concatenate([qpad, kpad, vpad], axis=1).astype(BF16)

    wpT = pk_branch(W_p, DP, DP_H, dp, 0)
    wkT = pk_branch(W_k, DK, DK_H, dk, 16)

    woutT = np.ascontiguousarray((W_out / 3.0).T).astype(BF16)
    bout = np.ascontiguousarray(b_out.reshape(DOUT, 1)).astype(np.float32)

    return {
        "xT": np.ascontiguousarray(xT),
        "maskT": np.ascontiguousarray(maskT),
        "waT": np.ascontiguousarray(waT),
        "wpT": np.ascontiguousarray(wpT),
        "wkT": np.ascontiguousarray(wkT),
        "woutT": woutT,
        "bout": bout,
    }


def kernel(x, W_a, W_p, W_k, W_out, b_out, mask):
    from concourse.bass_utils import run_bass_kernel_spmd

    x = np.asarray(x, np.float32)
    W_a = np.asarray(W_a, np.float32)
    W_p = np.asarray(W_p, np.float32)
    W_k = np.asarray(W_k, np.float32)
    W_out = np.asarray(W_out, np.float32)
    b_out = np.asarray(b_out, np.float32)
    mask = np.asarray(mask)

    if "nc" not in _CACHE:
        _CACHE["nc"] = _build()
    nc = _CACHE["nc"]

    in_maps = [_prep_core(c, x, W_a, W_p, W_k, W_out, b_out, mask)
               for c in range(NCORES)]
    res = run_bass_kernel_spmd(nc, in_maps, core_ids=list(range(NCORES)))

    outs = []
    for b in range(B):
        outs.append(np.asarray(res.results[2 * b]["out"]).astype(np.float32).T)
    return np.stack(outs, axis=0)


# revision 17
# speedup vs baseline: 1.4097x; 1.1341x over previous
"""Distributed Trainium2 kernel for the 3-branch masked attention problem.

Sharding: 8 cores; core c handles batch b = c//2 and heads h0 = 4*(c%2) .. +4
(data + head parallel).  Each core computes QKV for its heads, the three
branch softmaxes and AV locally, then a pair-wise AllGather of the [256, 2048]
attention output (transposed) lets both cores of a batch apply the output
projection.  Host-side work is limited to sharding/layout (transposes, bf16
conversion, constant folding of d**-0.5 and the 1/3 branch average).
"""

import numpy as np
import ml_dtypes

BF16 = ml_dtypes.bfloat16

H = 8
DA, DP, DK = 2048, 1024, 1024
B, N = 4, 2048
DOUT = 512
H_LOC = 4           # heads per core
DA_H, DP_H, DK_H = DA // H, DP // H, DK // H      # 256, 128, 128
da, dp, dk = DA_H // H, DP_H // H, DK_H // H      # 32, 16, 16
DV = da + dp + dk                                 # 64
NCORES = 8

IB = 512            # query block (moving dim of dots / AV)
JB = 128            # key chunk (contract chunk of AV, M of dots)
NI = N // IB        # 4
NJ = N // JB        # 16

_CACHE = {}


def _build():
    import concourse.bass as bass
    import concourse.mybir as mybir
    import concourse.tile as tile
    from concourse import bacc
    from concourse.masks import make_identity
    from concourse.tile import add_dep_helper

    f32 = mybir.dt.float32
    bf16 = mybir.dt.bfloat16
    Exp = mybir.ActivationFunctionType.Exp
    mult = mybir.AluOpType.mult
    add = mybir.AluOpType.add

    nc = bacc.Bacc("TRN2", target_bir_lowering=False, debug=False,
                   enable_asserts=False, num_devices=NCORES)

    xT = nc.dram_tensor("xT", [DA + DP + DK, N], bf16, kind="ExternalInput")
    maskT = nc.dram_tensor("maskT", [N, N], bf16, kind="ExternalInput")
    waT = nc.dram_tensor("waT", [DA, 384], bf16, kind="ExternalInput")
    wpT = nc.dram_tensor("wpT", [DP, 384], bf16, kind="ExternalInput")
    wkT = nc.dram_tensor("wkT", [DK, 384], bf16, kind="ExternalInput")
    woutT = nc.dram_tensor("woutT", [DOUT, DOUT], bf16, kind="ExternalInput")
    bout = nc.dram_tensor("bout", [DOUT, 1], f32, kind="ExternalInput")
    out = nc.dram_tensor("out", [DOUT, N], bf16, kind="ExternalOutput")

    with tile.TileContext(nc) as tc:
        with (
            tc.tile_pool(name="const", bufs=1) as cpool,
            tc.tile_pool(name="dram", bufs=1, space="DRAM") as dpool,
        ):
            # ---- constants ----
            ident_bf = cpool.tile([128, 128], bf16)
            make_identity(nc, ident_bf)
            ident_f32 = cpool.tile([128, 128], f32)
            make_identity(nc, ident_f32)

            bias_sb = cpool.tile([128, 4], f32)
            for t in range(4):
                nc.sync.dma_start(bias_sb[:, t:t + 1], bout[128 * t:128 * (t + 1), :])

            wa_sb = [cpool.tile([128, 384], bf16, name=f"wa{f}") for f in range(16)]
            for f in range(16):
                nc.sync.dma_start(wa_sb[f][:], waT[128 * f:128 * (f + 1), :])
            wp_sb = [cpool.tile([128, 384], bf16, name=f"wp{f}") for f in range(8)]
            wk_sb = [cpool.tile([128, 384], bf16, name=f"wk{f}") for f in range(8)]
            for f in range(8):
                nc.sync.dma_start(wp_sb[f][:], wpT[128 * f:128 * (f + 1), :])
                nc.sync.dma_start(wk_sb[f][:], wkT[128 * f:128 * (f + 1), :])
            wo_sb = [cpool.tile([128, DOUT], bf16, name=f"wo{f}") for f in range(4)]
            for f in range(4):
                nc.sync.dma_start(wo_sb[f][:], woutT[128 * f:128 * (f + 1), :])

            # ---- persistent activations ----
            # qT/kT per branch: [128, N]; heads live at 32-aligned partition bases
            qTa = cpool.tile([128, N], bf16)
            kTa = cpool.tile([128, N], bf16)
            qTp = cpool.tile([128, N], bf16)
            kTp = cpool.tile([128, N], bf16)
            qTk = cpool.tile([128, N], bf16)
            kTk = cpool.tile([128, N], bf16)
            # V^T combined: head h at rows 64*(h%2)+[va(32)|vp(16)|vk(16)] of tile h//2
            comb = [cpool.tile([128, N], bf16, name=f"comb{i}") for i in range(2)]
            # V_aug per head: 16 chunks of [128, 128] side by side: cols
            # 0:64 = v, col 64 = ones, 65:128 = zeros (M=128 keeps the PE
            # array fully configured so HAM doesn't throttle the clock)
            vaug = [cpool.tile([128, 128 * NJ], bf16, name=f"vaug{h}") for h in range(H_LOC)]
            # normalized attention output accumulator, [token, dv] layout
            oacc = [[cpool.tile([128, DV], f32, name=f"oacc{h}_{s}") for s in range(N // 128)]
                    for h in range(H_LOC)]
            # final transposed attention output (this core's heads)
            otc = [cpool.tile([128, N], bf16, name=f"otc{i}") for i in range(2)]

            # per-token-half collective bounce buffers: the first half's
            # AllGather overlaps the tail of attention
            cc_in_h = [dpool.tile([2 * 128, N // 2], bf16, name=f"ccin{T}")
                       for T in range(2)]
            cc_out_h = [dpool.tile([4 * 128, N // 2], bf16, name=f"ccout{T}")
                        for T in range(2)]

            for h in range(H_LOC):
                nc.gpsimd.memset(vaug[h][:], 0.0)
                for j in range(NJ):
                    nc.gpsimd.memset(vaug[h][:, 128 * j + 64:128 * j + 65], 1.0)

            _mctx = tc.tile_pool(name="mask", bufs=1)
            mpool = _mctx.__enter__()
            m_sb = [mpool.tile([128, N], bf16, name=f"m{j}") for j in range(NJ)]

            # =================== QKV projection ===================
            with (
                tc.tile_pool(name="xs", bufs=8) as xpool,
                tc.tile_pool(name="qkv_ps", bufs=6, space="PSUM") as qkv_ps,
                tc.tile_pool(name="vtr_ps", bufs=2, space="PSUM") as vtr_ps,
            ):
                # branch spec: (x row offset, n f-chunks, weights)
                # p and k share a v accumulator: vp at psum rows 32h+0:16,
                # vk at 32h+16:32 (host-padded weight columns) so every
                # PSUM read is 32-partition aligned.
                branches = [
                    (0, 16, wa_sb),
                    (DA, 8, wp_sb),
                    (DA + DP, 8, wk_sb),
                ]
                # pass 1: branch a complete (q, k, v in one x stream)
                for tp2 in range(2):
                    t0 = 2 * IB * tp2
                    ps_q = [qkv_ps.tile([128, IB], f32, tag="qkv", name=f"psq{u}")
                            for u in range(2)]
                    ps_k = [qkv_ps.tile([128, IB], f32, tag="qkv", name=f"psk{u}")
                            for u in range(2)]
                    ps_va = [qkv_ps.tile([128, IB], f32, tag="qkv", name=f"psva{u}")
                             for u in range(2)]
                    for f in range(16):
                        xt = xpool.tile([128, 2 * IB], bf16, tag="x")
                        nc.sync.dma_start(
                            xt[:], xT[128 * f:128 * (f + 1), t0:t0 + 2 * IB])
                        st, sp = (f == 0), (f == 15)
                        w = wa_sb[f]
                        for u in range(2):
                            xu = xt[:, IB * u:IB * (u + 1)]
                            nc.tensor.matmul(ps_q[u][:], w[:, 0:128], xu, start=st, stop=sp)
                            nc.tensor.matmul(ps_k[u][:], w[:, 128:256], xu, start=st, stop=sp)
                            nc.tensor.matmul(ps_va[u][:], w[:, 256:384], xu, start=st, stop=sp)
                    for u in range(2):
                        tsl = slice(t0 + IB * u, t0 + IB * (u + 1))
                        nc.vector.tensor_copy(qTa[:, tsl], ps_q[u][:])
                        nc.vector.tensor_copy(kTa[:, tsl], ps_k[u][:])
                        for h in range(H_LOC):
                            nc.vector.tensor_copy(
                                comb[h // 2][64 * (h % 2):64 * (h % 2) + da, tsl],
                                ps_va[u][da * h:da * (h + 1), :])

                # pass 2: p+k v projections (shared accumulator)
                for tp2 in range(2):
                    t0 = 2 * IB * tp2
                    ps_vpk = [qkv_ps.tile([128, IB], f32, tag="qkv", name=f"psvpk{u}")
                              for u in range(2)]
                    for bi in (1, 2):
                        fofs, nf, wsb = branches[bi]
                        for f in range(nf):
                            xt = xpool.tile([128, 2 * IB], bf16, tag="x")
                            nc.sync.dma_start(
                                xt[:], xT[fofs + 128 * f:fofs + 128 * (f + 1),
                                          t0:t0 + 2 * IB])
                            vst = (f == 0) and bi == 1
                            vsp = (f == nf - 1) and bi == 2
                            for u in range(2):
                                nc.tensor.matmul(ps_vpk[u][:], wsb[f][:, 256:384],
                                                 xt[:, IB * u:IB * (u + 1)],
                                                 start=vst, stop=vsp)
                    for u in range(2):
                        tsl = slice(t0 + IB * u, t0 + IB * (u + 1))
                        for h in range(H_LOC):
                            nc.vector.tensor_copy(
                                comb[h // 2][64 * (h % 2) + da:64 * (h % 2) + 64, tsl],
                                ps_vpk[u][32 * h:32 * (h + 1), :])

                # V_aug: transpose comb chunks
                for j in range(NJ):
                    jsl = slice(128 * j, 128 * (j + 1))
                    for c in range(2):
                        tp = vtr_ps.tile([128, 128], bf16, tag="vtr")
                        nc.tensor.transpose(tp[:], comb[c][:, jsl], ident_bf[:])
                        nc.vector.tensor_copy(vaug[2 * c][:, 128 * j:128 * j + 64], tp[:, 0:64])
                        nc.vector.tensor_copy(vaug[2 * c + 1][:, 128 * j:128 * j + 64], tp[:, 64:128])

                # mask tiles
                for j in range(NJ):
                    nc.sync.dma_start(m_sb[j][:],
                                      maskT[128 * j:128 * (j + 1), :])

                # pass 3: q/k for branches p and k
                for bi in (1, 2):
                    fofs, nf, wsb = branches[bi]
                    for tp2 in range(2):
                        t0 = 2 * IB * tp2
                        ps_q = [qkv_ps.tile([128, IB], f32, tag="qkv", name=f"psq{u}")
                                for u in range(2)]
                        ps_k = [qkv_ps.tile([128, IB], f32, tag="qkv", name=f"psk{u}")
                                for u in range(2)]
                        for f in range(nf):
                            xt = xpool.tile([128, 2 * IB], bf16, tag="x")
                            nc.sync.dma_start(
                                xt[:], xT[fofs + 128 * f:fofs + 128 * (f + 1),
                                          t0:t0 + 2 * IB])
                            st, sp = (f == 0), (f == nf - 1)
                            w = wsb[f]
                            for u in range(2):
                                nc.tensor.matmul(ps_q[u][:], w[:, 0:128],
                                                 xt[:, IB * u:IB * (u + 1)],
                                                 start=st, stop=sp)
                                nc.tensor.matmul(ps_k[u][:], w[:, 128:256],
                                                 xt[:, IB * u:IB * (u + 1)],
                                                 start=st, stop=sp)
                        qT_t = (qTa, qTp, qTk)[bi]
                        kT_t = (kTa, kTp, kTk)[bi]
                        for u in range(2):
                            tsl = slice(t0 + IB * u, t0 + IB * (u + 1))
                            nc.vector.tensor_copy(qT_t[:, tsl], ps_q[u][:])
                            nc.vector.tensor_copy(kT_t[:, tsl], ps_k[u][:])
            # =================== attention ===================
            with (
                tc.tile_pool(name="otf", bufs=1) as otfpool,
                tc.tile_pool(name="s_ps", bufs=2, space="PSUM") as s_ps_pool,
                tc.tile_pool(name="o_ps", bufs=4, space="PSUM") as o_ps_pool,
                tc.tile_pool(name="ep", bufs=5) as epool,
                tc.tile_pool(name="pp", bufs=5) as ppool,
                tc.tile_pool(name="ob", bufs=3) as opool,
                tc.tile_pool(name="rr", bufs=4) as rpool,
            ):
                otf_h = [[otfpool.tile([128, N // 2], bf16, name=f"otf{T}_{c}")
                          for c in range(4)] for T in range(2)]
                battn = [(qTa, kTa, da), (qTp, kTp, dp), (qTk, kTk, dk)]
                for bi, (qT_t, kT_t, d) in enumerate(battn):
                    for I in range(NI):
                        isl = slice(IB * I, IB * (I + 1))
                        # 2 halves of 2 heads each: each half has its own
                        # 2-bank S tile, so dots of one half overlap exp
                        # of the other; within a half the 2 row-disjoint
                        # dots are chained adjacent to run concurrently
                        o_ps_h = [o_ps_pool.tile([128, IB], f32, tag="o",
                                                 name=f"ops{h}")
                                  for h in range(H_LOC)]

                        def emit_av(jj, hf, pp_sb):
                            for hh in range(2):
                                h = 2 * hf + hh
                                nc.tensor.matmul(
                                    o_ps_h[h][:],
                                    vaug[h][:, 128 * jj:128 * (jj + 1)],
                                    pp_sb[:, IB * hh:IB * (hh + 1)],
                                    start=(jj == 0), stop=(jj == NJ - 1),
                                    skip_group_check=True)

                        # while DVE digests the previous block's epilogue
                        # (first ~5 j's), mask-multiplies go to gpsimd and
                        # their AVs are emitted two j's later so the slow
                        # gpsimd op can't head-of-line-block the PE stream
                        first_blk = (bi == 0 and I == 0)
                        av_backlog = []
                        for j in range(NJ):
                            for half in range(2):
                                s_ps = s_ps_pool.tile([128, 2 * IB], f32,
                                                      tag="s", name=f"sh{half}")
                                dots = []
                                for hh in range(2):
                                    h = 2 * half + hh
                                    pb = 32 * h
                                    mm = nc.tensor.matmul(
                                        s_ps[:, IB * hh:IB * (hh + 1)],
                                        kT_t[pb:pb + d, 128 * j:128 * (j + 1)],
                                        qT_t[pb:pb + d, isl],
                                        start=True, stop=True,
                                        tile_position=(pb, 0))
                                    if dots:
                                        add_dep_helper(mm.ins, dots[-1].ins,
                                                       sync=False,
                                                       reason="chain dots")
                                    dots.append(mm)
                                if half == 0:
                                    while av_backlog and av_backlog[0][0] <= j:
                                        _, jj, hf, pp_sb = av_backlog.pop(0)
                                        emit_av(jj, hf, pp_sb)
                                e_sb = epool.tile([128, 2 * IB], bf16, tag="e")
                                nc.scalar.activation(e_sb[:], s_ps[:], Exp)
                                p_sb = ppool.tile([128, 2 * IB], bf16, tag="p")
                                m_bc = m_sb[j][:, None, isl].broadcast_to(
                                    [128, 2, IB])
                                on_gp = False
                                teng = nc.vector
                                teng.tensor_tensor(
                                    p_sb[:].rearrange("p (g i) -> p g i", g=2),
                                    e_sb[:].rearrange("p (g i) -> p g i", g=2),
                                    m_bc, op=mult)
                                if on_gp:
                                    av_backlog.append((j + 2, j, half, p_sb))
                                else:
                                    emit_av(j, half, p_sb)
                        for _, jj, hf, pp_sb in av_backlog:
                            emit_av(jj, hf, pp_sb)
                        # epilogue: drain all four accumulators first so the
                        # next block's AVs get PSUM slots immediately, then
                        # normalize + accumulate
                        o_sbs = []
                        for h in range(H_LOC):
                            o_sb = opool.tile([65, IB], f32, tag="osb",
                                              name=f"osb{h}")
                            nc.vector.tensor_copy(o_sb[:], o_ps_h[h][0:65, :])
                            o_sbs.append(o_sb)
                        for h in range(H_LOC):
                            for s in range(IB // 128):
                                tp = o_ps_pool.tile([128, 65], f32, tag="o",
                                                    name="tps")
                                nc.tensor.transpose(
                                    tp[:], o_sbs[h][:, 128 * s:128 * (s + 1)],
                                    ident_f32[0:65, 0:65])
                                r_sb = rpool.tile([128, 1], f32, tag="r")
                                nc.vector.reciprocal(r_sb[:], tp[:, 64:65])
                                at = oacc[h][4 * I + s]
                                if bi == 0:
                                    nc.vector.tensor_scalar_mul(at[:], tp[:, 0:DV], r_sb[:])
                                else:
                                    nc.vector.scalar_tensor_tensor(
                                        at[:], tp[:, 0:DV], r_sb[:], at[:],
                                        op0=mult, op1=add)
                                if bi == 2:
                                    tp2 = o_ps_pool.tile([DV, 128], f32,
                                                         tag="o", name="t2")
                                    nc.tensor.transpose(tp2[:], at[:],
                                                        ident_f32[:])
                                    sl = 4 * I + s
                                    nc.vector.tensor_copy(
                                        otc[h // 2][64 * (h % 2):64 * (h % 2) + DV,
                                                    128 * sl:128 * (sl + 1)],
                                        tp2[:])
                        if bi == 2 and I in (1, 3):
                            # token half T of the attention output is final:
                            # gather it across the core pair now so the
                            # collective+DMA latency hides under attention
                            T = I // 2
                            hsl = slice(1024 * T, 1024 * (T + 1))
                            for c in range(2):
                                nc.sync.dma_start(
                                    cc_in_h[T][128 * c:128 * (c + 1), :],
                                    otc[c][:, hsl])
                            nc.gpsimd.collective_compute(
                                "AllGather",
                                mybir.AluOpType.bypass,
                                replica_groups=[[0, 1], [2, 3], [4, 5], [6, 7]],
                                ins=[cc_in_h[T].opt()],
                                outs=[cc_out_h[T].opt()],
                            )
                            for c in range(4):
                                nc.sync.dma_start(
                                    otf_h[T][c][:],
                                    cc_out_h[T][128 * c:128 * (c + 1), :])

                # =================== output projection ===================
                for T in range(2):
                    for ot in range(4):
                        for i2 in range(2):
                            i2sl = slice(512 * i2, 512 * (i2 + 1))
                            ps = o_ps_pool.tile([128, 512], f32, tag="o",
                                                name="fps")
                            for ic in range(4):
                                nc.tensor.matmul(
                                    ps[:], wo_sb[ic][:, 128 * ot:128 * (ot + 1)],
                                    otf_h[T][ic][:, i2sl],
                                    start=(ic == 0), stop=(ic == 3))
                            fin = epool.tile([128, 512], bf16, tag="e",
                                             name="fin")
                            nc.vector.tensor_scalar_add(fin[:], ps[:],
                                                        bias_sb[:, ot:ot + 1])
                            nc.sync.dma_start(
                                out[128 * ot:128 * (ot + 1),
                                    1024 * T + 512 * i2:1024 * T + 512 * (i2 + 1)],
                                fin[:])

            _mctx.__exit__(None, None, None)

    nc.compile()
    return nc


def _prep_core(c, x, W_a, W_p, W_k, W_out, b_out, mask):
    b = c // 2
    h0 = H_LOC * (c % 2)

    xT = np.ascontiguousarray(x[b].T).astype(BF16)
    maskT = np.ascontiguousarray(mask[b, 0].T).astype(BF16)

    qa = W_a[da * h0: da * (h0 + H_LOC), :] * (DA ** -0.5)
    ka = W_a[DA_H + da * h0: DA_H + da * (h0 + H_LOC), :]
    va = W_a[2 * DA_H + da * h0: 2 * DA_H + da * (h0 + H_LOC), :]
    waT = np.concatenate([qa.T, ka.T, va.T], axis=1).astype(BF16)

    def pk_branch(W, D, D_H, d, vcol_ofs):
        qpad = np.zeros((D, 128), np.float32)
        kpad = np.zeros((D, 128), np.float32)
        vpad = np.zeros((D, 128), np.float32)
        for h in range(H_LOC):
            qpad[:, 32 * h:32 * h + d] = W[d * (h0 + h): d * (h0 + h + 1), :].T * (D ** -0.5)
            kpad[:, 32 * h:32 * h + d] = W[D_H + d * (h0 + h): D_H + d * (h0 + h + 1), :].T
            vpad[:, 32 * h + vcol_ofs:32 * h + vcol_ofs + d] = \
                W[2 * D_H + d * (h0 + h): 2 * D_H + d * (h0 + h + 1), :].T
        return np.concatenate([qpad, kpad, vpad], axis=1).astype(BF16)

    wpT = pk_branch(W_p, DP, DP_H, dp, 0)
    wkT = pk_branch(W_k, DK, DK_H, dk, 16)

    woutT = np.ascontiguousarray((W_out / 3.0).T).astype(BF16)
    bout = np.ascontiguousarray(b_out.reshape(DOUT, 1)).astype(np.float32)

    return {
        "xT": np.ascontiguousarray(xT),
        "maskT": np.ascontiguousarray(maskT),
        "waT": np.ascontiguousarray(waT),
        "wpT": np.ascontiguousarray(wpT),
        "wkT": np.ascontiguousarray(wkT),
        "woutT": woutT,
        "bout": bout,
    }


def kernel(x, W_a, W_p, W_k, W_out, b_out, mask):
    from concourse.bass_utils import run_bass_kernel_spmd

    x = np.asarray(x, np.float32)
    W_a = np.asarray(W_a, np.float32)
    W_p = np.asarray(W_p, np.float32)
    W_k = np.asarray(W_k, np.float32)
    W_out = np.asarray(W_out, np.float32)
    b_out = np.asarray(b_out, np.float32)
    mask = np.asarray(mask)

    if "nc" not in _CACHE:
        _CACHE["nc"] = _build()
    nc = _CACHE["nc"]

    in_maps = [_prep_core(c, x, W_a, W_p, W_k, W_out, b_out, mask)
               for c in range(NCORES)]
    res = run_bass_kernel_spmd(nc, in_maps, core_ids=list(range(NCORES)))

    outs = []
    for b in range(B):
        outs.append(np.asarray(res.results[2 * b]["out"]).astype(np.float32).T)
    return np.stack(outs, axis=0)

